# revision 4
# baseline (speedup 1.0000x reference)
"""BoneLinear Trainium2 kernel (8-core SPMD, data-parallel over batch).

Math: reference computes out = x @ (weight + w)^T where w is the bone
block-update of weight:
    wblk = weight.reshape(a, r, b, r).transpose(0,2,1,3)      # (a,b,r,r)
    wup  = wblk @ bone[b] + bone[b]                            # per (a,b)
    w    = wup.transpose(0,2,1,3).reshape(out_f, in_f)

Identity used here (verified numerically): with y[:, b*r:(b+1)*r] =
x[:, b*r:(b+1)*r] @ bone[b].T and s = sum_b y[:, b-block]:

    out = (x + y) @ weight^T + tile(s over out-blocks)

so the heavy GEMM uses the *original* weight; the bone update reduces to a
cheap block-diagonal transform of x plus a rank-64 broadcast correction.

Per core (batch element): z^T = x^T + blockdiag(bone^T) @ x^T is computed on
the PE in t-quarters and kept SBUF-resident in fp16; the main GEMM
out = z^T.T @ W^T streams W^T from HBM; s is accumulated on the PE and added
(broadcast over 64-column blocks) during PSUM eviction.
"""

import numpy as np

B, T, IN, OUT, R = 8, 2048, 4096, 4096, 64
P = 128
KT = IN // P  # 32 contraction tiles
TQ = 512  # t-quarter size
NQ = T // TQ  # 4 quarters
NFREE = 512  # matmul moving free dim / o-tile size
OTN = OUT // NFREE  # 8 o-tiles

_NC_CACHE = {}


def _build_nc(reps=1):
    import concourse.mybir as mybir
    from concourse import bacc
    from concourse.tile import TileContext
    from concourse.masks import make_identity

    F16 = mybir.dt.float16
    F32 = mybir.dt.float32

    nc = bacc.Bacc(None, target_bir_lowering=False)
    xT = nc.dram_tensor("xT", [IN, T], F16, kind="ExternalInput")
    wT = nc.dram_tensor("wT", [IN, OUT], F16, kind="ExternalInput")
    bd = nc.dram_tensor("bd", [P, KT, P], F16, kind="ExternalInput")
    bdv = nc.dram_tensor("bdv", [P, KT, R], F16, kind="ExternalInput")
    out = nc.dram_tensor("out", [T, OUT], F32, kind="ExternalOutput")

    wTv = wT.rearrange("(ko p) o -> p ko o", p=P)

    with TileContext(nc) as tc:
        with (
            tc.tile_pool(name="const", bufs=1) as constp,
            tc.tile_pool(name="xt", bufs=3) as xpool,
            tc.tile_pool(name="zt", bufs=2) as zpool,
            tc.tile_pool(name="wt", bufs=2) as wpool,
            tc.tile_pool(name="sb", bufs=2) as spool,
            tc.tile_pool(name="ob", bufs=3) as opool,
            tc.tile_pool(name="py", bufs=2, space="PSUM") as pyp,
            tc.tile_pool(name="ps", bufs=1, space="PSUM") as psp,
            tc.tile_pool(name="po", bufs=4, space="PSUM") as pop,
        ):
            bd_sb = constp.tile([P, KT, P], F16, tag="bd")
            nc.sync.dma_start(bd_sb[:], bd[:])
            bdv_sb = constp.tile([P, KT, R], F16, tag="bdv")
            nc.sync.dma_start(bdv_sb[:], bdv[:])
            ident = constp.tile([R, R], F32, tag="ident")
            make_identity(nc, ident)

            # reps>1 repeats the whole computation (timing builds only —
            # wall-time differencing cancels host/transfer overhead).
            for q in range(NQ * reps):
                q = q % NQ
                t0 = q * TQ
                # ---- phase 1: z^T quarter + s accumulation ----
                zt = zpool.tile([P, KT, TQ], F16, tag="zt")
                psum_s = psp.tile([R, TQ], F32, tag="ps")
                for k in range(KT):
                    xt = xpool.tile([P, TQ], F16, tag="xt")
                    nc.sync.dma_start(xt[:], xT[k * P : (k + 1) * P, t0 : t0 + TQ])
                    py = pyp.tile([P, TQ], F32, tag="py")
                    # y^T tile: blockdiag(bone[2k]^T, bone[2k+1]^T)^T-mm
                    nc.tensor.matmul(
                        py[:], bd_sb[:, k, :], xt[:], start=True, stop=True
                    )
                    # s^T accumulation: vstack(bone[2k]^T, bone[2k+1]^T)
                    nc.tensor.matmul(
                        psum_s[:],
                        bdv_sb[:, k, :],
                        xt[:],
                        start=(k == 0),
                        stop=(k == KT - 1),
                    )
                    nc.vector.tensor_add(zt[:, k, :], py[:], xt[:])
                # s: [R, TQ] -> t-partition layout [P, TQ//P, R]
                sT = spool.tile([R, TQ], F32, tag="sT")
                nc.vector.tensor_copy(sT[:], psum_s[:])
                s_sb = spool.tile([P, TQ // P, R], F32, tag="ssb")
                for c in range(TQ // P):
                    pt = pyp.tile([P, R], F32, tag="py")
                    nc.tensor.transpose(pt[:], sT[:, c * P : (c + 1) * P], ident[:])
                    nc.vector.tensor_copy(s_sb[:, c, :], pt[:])

                # ---- phase 2: out quarter = z^T.T @ W^T + s ----
                for ot in range(OTN):
                    wt = wpool.tile([P, KT, NFREE], F16, tag="wt")
                    nc.sync.dma_start(wt[:], wTv[:, :, ot * NFREE : (ot + 1) * NFREE])
                    for tt in range(TQ // P):
                        po = pop.tile([P, NFREE], F32, tag="po")
                        for k in range(KT):
                            nc.tensor.matmul(
                                po[:],
                                zt[:, k, tt * P : (tt + 1) * P],
                                wt[:, k, :],
                                start=(k == 0),
                                stop=(k == KT - 1),
                            )
                        ob = opool.tile([P, NFREE], F32, tag="ob")
                        ob3 = ob.rearrange("p (a r) -> p a r", r=R)
                        po3 = po.rearrange("p (a r) -> p a r", r=R)
                        s_bcast = s_sb[:, tt, :][:, None, :].to_broadcast(
                            (P, NFREE // R, R)
                        )
                        nc.vector.tensor_add(ob3, po3, s_bcast)
                        nc.sync.dma_start(
                            out[
                                t0 + tt * P : t0 + (tt + 1) * P,
                                ot * NFREE : (ot + 1) * NFREE,
                            ],
                            ob[:],
                        )
    nc.compile()
    return nc


def _get_nc(reps=1):
    key = ("nc", reps)
    if key not in _NC_CACHE:
        _NC_CACHE[key] = _build_nc(reps)
    return _NC_CACHE[key]


def kernel(x, weight, bone):
    from concourse.bass_utils import run_bass_kernel_spmd

    x = np.asarray(x)
    weight = np.asarray(weight)
    bone = np.asarray(bone)

    nc = _get_nc()

    # Layout prep (host): transposes + block placement only.
    wT16 = np.ascontiguousarray(weight.T).astype(np.float16)
    boneT = bone.transpose(0, 2, 1).astype(np.float16)  # bone[b]^T
    bdmat = np.zeros((KT, P, P), np.float16)
    bdmat[:, 0:R, 0:R] = boneT[0::2]
    bdmat[:, R:P, R:P] = boneT[1::2]
    bd_host = np.ascontiguousarray(bdmat.transpose(1, 0, 2))  # [P, KT, P]
    bdvm = np.zeros((KT, P, R), np.float16)
    bdvm[:, 0:R, :] = boneT[0::2]
    bdvm[:, R:P, :] = boneT[1::2]
    bdv_host = np.ascontiguousarray(bdvm.transpose(1, 0, 2))  # [P, KT, R]

    in_maps = []
    for i in range(B):
        xT16 = np.ascontiguousarray(x[i].T).astype(np.float16)
        in_maps.append({"xT": xT16, "wT": wT16, "bd": bd_host, "bdv": bdv_host})

    res = run_bass_kernel_spmd(nc, in_maps, core_ids=list(range(B)))
    return np.stack([r["out"] for r in res.results], axis=0)


if __name__ == "__main__":
    rng = np.random.default_rng(0)
    x = rng.standard_normal((B, T, IN), dtype=np.float32)
    weight = (rng.standard_normal((OUT, IN)) * 0.02).astype(np.float32)
    bone = (rng.standard_normal((IN // R, R, R)) * 0.02).astype(np.float32)
    out = kernel(x=x, weight=weight, bone=bone)
    print(out.shape, out.dtype)


# revision 5
# speedup vs baseline: 3.6489x; 3.6489x over previous
"""BoneLinear Trainium2 kernel (8-core SPMD, data-parallel over batch).

Math: reference computes out = x @ (weight + w)^T where w is the bone
block-update of weight:
    wblk = weight.reshape(a, r, b, r).transpose(0,2,1,3)      # (a,b,r,r)
    wup  = wblk @ bone[b] + bone[b]                            # per (a,b)
    w    = wup.transpose(0,2,1,3).reshape(out_f, in_f)

Identity used here (verified numerically): with y[:, b*r:(b+1)*r] =
x[:, b*r:(b+1)*r] @ bone[b].T and s = sum_b y[:, b-block]:

    out = (x + y) @ weight^T + tile(s over out-blocks)

so the heavy GEMM uses the *original* weight; the bone update reduces to a
cheap block-diagonal transform of x plus a rank-64 broadcast correction.

Per core (batch element): z^T = x^T + blockdiag(bone^T) @ x^T is computed on
the PE in t-quarters and kept SBUF-resident in fp16; the main GEMM
out = z^T.T @ W^T streams W^T from HBM; s is accumulated on the PE and added
(broadcast over 64-column blocks) during PSUM eviction.
"""

import numpy as np

B, T, IN, OUT, R = 8, 2048, 4096, 4096, 64
P = 128
KT = IN // P  # 32 contraction tiles
TQ = 512  # t-quarter size
NQ = T // TQ  # 4 quarters
NFREE = 512  # matmul moving free dim / o-tile size
OTN = OUT // NFREE  # 8 o-tiles

_NC_CACHE = {}


def _build_nc(reps=1):
    import concourse.mybir as mybir
    from concourse import bacc
    from concourse.tile import TileContext
    from concourse.masks import make_identity

    F16 = mybir.dt.float16
    F32 = mybir.dt.float32

    nc = bacc.Bacc(None, target_bir_lowering=False)
    xT = nc.dram_tensor("xT", [IN, T], F16, kind="ExternalInput")
    wT = nc.dram_tensor("wT", [IN, OUT], F16, kind="ExternalInput")
    bd = nc.dram_tensor("bd", [P, KT, P], F16, kind="ExternalInput")
    bdv = nc.dram_tensor("bdv", [P, KT, R], F16, kind="ExternalInput")
    out = nc.dram_tensor("out", [T, OUT], F32, kind="ExternalOutput")

    wTv = wT.rearrange("(ko p) o -> p ko o", p=P)

    with TileContext(nc) as tc:
        with (
            tc.tile_pool(name="const", bufs=1) as constp,
            tc.tile_pool(name="xt", bufs=3) as xpool,
            tc.tile_pool(name="zt", bufs=2) as zpool,
            tc.tile_pool(name="wt", bufs=2) as wpool,
            tc.tile_pool(name="sb", bufs=2) as spool,
            tc.tile_pool(name="ob", bufs=3) as opool,
            tc.tile_pool(name="py", bufs=2, space="PSUM") as pyp,
            tc.tile_pool(name="ps", bufs=1, space="PSUM") as psp,
            tc.tile_pool(name="po", bufs=4, space="PSUM") as pop,
        ):
            bd_sb = constp.tile([P, KT, P], F16, tag="bd")
            nc.sync.dma_start(bd_sb[:], bd[:])
            bdv_sb = constp.tile([P, KT, R], F16, tag="bdv")
            nc.sync.dma_start(bdv_sb[:], bdv[:])
            ident = constp.tile([R, R], F32, tag="ident")
            make_identity(nc, ident)

            # reps>1 repeats the whole computation (timing builds only —
            # wall-time differencing cancels host/transfer overhead).
            for q in range(NQ * reps):
                q = q % NQ
                t0 = q * TQ
                # ---- phase 1: z^T quarter + s accumulation ----
                zt = zpool.tile([P, KT, TQ], F16, tag="zt")
                psum_s = psp.tile([R, TQ], F32, tag="ps")
                for k in range(KT):
                    xt = xpool.tile([P, TQ], F16, tag="xt")
                    nc.sync.dma_start(xt[:], xT[k * P : (k + 1) * P, t0 : t0 + TQ])
                    py = pyp.tile([P, TQ], F32, tag="py")
                    # y^T tile: blockdiag(bone[2k]^T, bone[2k+1]^T)^T-mm
                    nc.tensor.matmul(
                        py[:], bd_sb[:, k, :], xt[:], start=True, stop=True
                    )
                    # s^T accumulation: vstack(bone[2k]^T, bone[2k+1]^T)
                    nc.tensor.matmul(
                        psum_s[:],
                        bdv_sb[:, k, :],
                        xt[:],
                        start=(k == 0),
                        stop=(k == KT - 1),
                    )
                    nc.vector.tensor_add(zt[:, k, :], py[:], xt[:])
                # s: [R, TQ] -> t-partition layout [P, TQ//P, R]
                sT = spool.tile([R, TQ], F32, tag="sT")
                nc.vector.tensor_copy(sT[:], psum_s[:])
                s_sb = spool.tile([P, TQ // P, R], F32, tag="ssb")
                for c in range(TQ // P):
                    pt = pyp.tile([P, R], F32, tag="py")
                    nc.tensor.transpose(pt[:], sT[:, c * P : (c + 1) * P], ident[:])
                    nc.vector.tensor_copy(s_sb[:, c, :], pt[:])

                # ---- phase 2: out quarter = z^T.T @ W^T + s ----
                for ot in range(OTN):
                    wt = wpool.tile([P, KT, NFREE], F16, tag="wt")
                    nc.sync.dma_start(wt[:], wTv[:, :, ot * NFREE : (ot + 1) * NFREE])
                    for tt in range(TQ // P):
                        po = pop.tile([P, NFREE], F32, tag="po")
                        for k in range(KT):
                            nc.tensor.matmul(
                                po[:],
                                zt[:, k, tt * P : (tt + 1) * P],
                                wt[:, k, :],
                                start=(k == 0),
                                stop=(k == KT - 1),
                            )
                        ob = opool.tile([P, NFREE], F32, tag="ob")
                        ob3 = ob.rearrange("p (a r) -> p a r", r=R)
                        po3 = po.rearrange("p (a r) -> p a r", r=R)
                        s_bcast = s_sb[:, tt, :][:, None, :].to_broadcast(
                            (P, NFREE // R, R)
                        )
                        nc.vector.tensor_add(ob3, po3, s_bcast)
                        nc.sync.dma_start(
                            out[
                                t0 + tt * P : t0 + (tt + 1) * P,
                                ot * NFREE : (ot + 1) * NFREE,
                            ],
                            ob[:],
                        )
    nc.compile()
    return nc


def _get_nc(reps=1):
    key = ("nc", reps)
    if key not in _NC_CACHE:
        _NC_CACHE[key] = _build_nc(reps)
    return _NC_CACHE[key]


def prep_in_maps(x, weight, bone):
    """Host-side layout prep: transposes + block placement + fp16 cast."""
    x = np.asarray(x)
    weight = np.asarray(weight)
    bone = np.asarray(bone)

    wT16 = np.ascontiguousarray(weight.T).astype(np.float16)
    boneT = bone.transpose(0, 2, 1).astype(np.float16)  # bone[b]^T
    bdmat = np.zeros((KT, P, P), np.float16)
    bdmat[:, 0:R, 0:R] = boneT[0::2]
    bdmat[:, R:P, R:P] = boneT[1::2]
    bd_host = np.ascontiguousarray(bdmat.transpose(1, 0, 2))  # [P, KT, P]
    bdvm = np.zeros((KT, P, R), np.float16)
    bdvm[:, 0:R, :] = boneT[0::2]
    bdvm[:, R:P, :] = boneT[1::2]
    bdv_host = np.ascontiguousarray(bdvm.transpose(1, 0, 2))  # [P, KT, R]

    in_maps = []
    for i in range(B):
        xT16 = np.ascontiguousarray(x[i].T).astype(np.float16)
        in_maps.append({"xT": xT16, "wT": wT16, "bd": bd_host, "bdv": bdv_host})
    return in_maps


def kernel(x, weight, bone):
    from concourse.bass_utils import run_bass_kernel_spmd

    nc = _get_nc()
    in_maps = prep_in_maps(x, weight, bone)
    res = run_bass_kernel_spmd(nc, in_maps, core_ids=list(range(B)))
    return np.stack([r["out"] for r in res.results], axis=0)


if __name__ == "__main__":
    rng = np.random.default_rng(0)
    x = rng.standard_normal((B, T, IN), dtype=np.float32)
    weight = (rng.standard_normal((OUT, IN)) * 0.02).astype(np.float32)
    bone = (rng.standard_normal((IN // R, R, R)) * 0.02).astype(np.float32)
    out = kernel(x=x, weight=weight, bone=bone)
    print(out.shape, out.dtype)


# revision 15
# speedup vs baseline: 3.8757x; 1.0622x over previous
"""BoneLinear Trainium2 kernel (8-core SPMD, data-parallel over batch).

Math: reference computes out = x @ (weight + w)^T where w is the bone
block-update of weight:
    wblk = weight.reshape(a, r, b, r).transpose(0,2,1,3)      # (a,b,r,r)
    wup  = wblk @ bone[b] + bone[b]                            # per (a,b)
    w    = wup.transpose(0,2,1,3).reshape(out_f, in_f)

Identity used here (verified numerically): with y[:, b*r:(b+1)*r] =
x[:, b*r:(b+1)*r] @ bone[b].T and s = sum_b y[:, b-block]:

    out = (x + y) @ weight^T + tile(s over out-blocks)

so the heavy GEMM uses the *original* weight; the bone update reduces to a
cheap block-diagonal transform of x plus a rank-64 broadcast correction.

Per core (batch element): z^T = x^T + blockdiag(bone^T) @ x^T is computed on
the PE in t-quarters and kept SBUF-resident in fp16; the main GEMM
out = z^T.T @ W^T streams W^T from HBM; s is accumulated on the PE and added
(broadcast over 64-column blocks) during PSUM eviction.
"""

import numpy as np

B, T, IN, OUT, R = 8, 2048, 4096, 4096, 64
P = 128
KT = IN // P  # 32 contraction tiles
TQ = 512  # t-quarter size
NQ = T // TQ  # 4 quarters
NFREE = 512  # matmul moving free dim / o-tile size
OTN = OUT // NFREE  # 8 o-tiles

_NC_CACHE = {}


def _build_nc(reps=1, nfree=NFREE, po_bufs=4, py_bufs=2, xt_bufs=36, wt_on_act=True):
    import concourse.mybir as mybir
    from concourse import bacc
    from concourse.tile import TileContext
    from concourse.masks import make_identity

    F16 = mybir.dt.float16
    F32 = mybir.dt.float32
    otn = OUT // nfree

    nc = bacc.Bacc(None, target_bir_lowering=False)
    xT = nc.dram_tensor("xT", [IN, T], F16, kind="ExternalInput")
    wT = nc.dram_tensor("wT", [IN, OUT], F16, kind="ExternalInput")
    bd = nc.dram_tensor("bd", [P, KT, P], F16, kind="ExternalInput")
    bdv = nc.dram_tensor("bdv", [P, KT, R], F16, kind="ExternalInput")
    out = nc.dram_tensor("out", [T, OUT], F32, kind="ExternalOutput")

    wTv = wT.rearrange("(ko p) o -> p ko o", p=P)

    with TileContext(nc) as tc:
        with (
            tc.tile_pool(name="const", bufs=1) as constp,
            tc.tile_pool(name="xt", bufs=xt_bufs) as xpool,
            tc.tile_pool(name="zt", bufs=2) as zpool,
            tc.tile_pool(name="wt", bufs=2) as wpool,
            tc.tile_pool(name="sb", bufs=2) as spool,
            tc.tile_pool(name="ob", bufs=3) as opool,
            tc.tile_pool(name="py", bufs=py_bufs, space="PSUM") as pyp,
            tc.tile_pool(name="ps", bufs=1, space="PSUM") as psp,
            tc.tile_pool(name="po", bufs=po_bufs, space="PSUM") as pop,
        ):
            bd_sb = constp.tile([P, KT, P], F16, tag="bd")
            nc.sync.dma_start(bd_sb[:], bd[:])
            bdv_sb = constp.tile([P, KT, R], F16, tag="bdv")
            nc.sync.dma_start(bdv_sb[:], bdv[:])
            ident = constp.tile([R, R], F32, tag="ident")
            make_identity(nc, ident)

            # Next-quarter xt tiles are prefetched during the current
            # quarter's phase 2, so phase 1 never waits on DMA (and the small
            # xt loads don't get stuck behind a 4MB wt transfer at the
            # quarter boundary).
            xt_tiles = {}

            def prefetch_xt(qi):
                tq0 = (qi % NQ) * TQ
                tiles = []
                for k in range(KT):
                    xt = xpool.tile([P, TQ], F16, tag="xt")
                    nc.sync.dma_start(
                        xt[:], xT[k * P : (k + 1) * P, tq0 : tq0 + TQ]
                    )
                    tiles.append(xt)
                xt_tiles[qi] = tiles

            # reps>1 repeats the whole computation (timing builds only —
            # wall-time differencing cancels host/transfer overhead).
            for qi in range(NQ * reps):
                q = qi % NQ
                t0 = q * TQ
                if qi == 0:
                    prefetch_xt(0)
                # ---- phase 1: z^T quarter + s accumulation ----
                zt = zpool.tile([P, KT, TQ], F16, tag="zt")
                psum_s = psp.tile([R, TQ], F32, tag="ps")
                xts = xt_tiles.pop(qi)
                for k in range(KT):
                    xt = xts[k]
                    py = pyp.tile([P, TQ], F32, tag="py")
                    # y^T tile: blockdiag(bone[2k]^T, bone[2k+1]^T)^T-mm
                    nc.tensor.matmul(
                        py[:], bd_sb[:, k, :], xt[:], start=True, stop=True
                    )
                    # s^T accumulation: vstack(bone[2k]^T, bone[2k+1]^T)
                    nc.tensor.matmul(
                        psum_s[:],
                        bdv_sb[:, k, :],
                        xt[:],
                        start=(k == 0),
                        stop=(k == KT - 1),
                    )
                    nc.vector.tensor_add(zt[:, k, :], py[:], xt[:])
                # s: [R, TQ] -> t-partition layout [P, TQ//P, R]
                sT = spool.tile([R, TQ], F32, tag="sT")
                nc.vector.tensor_copy(sT[:], psum_s[:])
                s_sb = spool.tile([P, TQ // P, R], F32, tag="ssb")
                for c in range(TQ // P):
                    pt = pyp.tile([P, R], F32, tag="py")
                    nc.tensor.transpose(pt[:], sT[:, c * P : (c + 1) * P], ident[:])
                    nc.vector.tensor_copy(s_sb[:, c, :], pt[:])

                # ---- phase 2: out quarter = z^T.T @ W^T + s ----
                # wt DMAs ride the ACT HWDGE ring (wt_on_act) so the next
                # quarter's first weight tile isn't FIFO-queued behind the 32
                # xt loads on the SP ring — hides the 4MB load under compute.
                wt_dma = nc.scalar.dma_start if wt_on_act else nc.sync.dma_start
                for ot in range(otn):
                    wt = wpool.tile([P, KT, nfree], F16, tag="wt")
                    # Very first weight tile rides the SP ring so it queues
                    # behind the startup xt loads instead of preempting them.
                    dma = nc.sync.dma_start if (qi == 0 and ot == 0) else wt_dma
                    dma(wt[:], wTv[:, :, ot * nfree : (ot + 1) * nfree])
                    if ot == 0 and qi + 1 < NQ * reps:
                        # Next quarter's x loads: emitted after this quarter's
                        # first weight tile so the SP ring serves wt first.
                        prefetch_xt(qi + 1)
                    for tt in range(TQ // P):
                        po = pop.tile([P, nfree], F32, tag="po")
                        for k in range(KT):
                            nc.tensor.matmul(
                                po[:],
                                zt[:, k, tt * P : (tt + 1) * P],
                                wt[:, k, :],
                                start=(k == 0),
                                stop=(k == KT - 1),
                            )
                        ob = opool.tile([P, nfree], F32, tag="ob")
                        ob3 = ob.rearrange("p (a r) -> p a r", r=R)
                        po3 = po.rearrange("p (a r) -> p a r", r=R)
                        s_bcast = s_sb[:, tt, :][:, None, :].to_broadcast(
                            (P, nfree // R, R)
                        )
                        nc.vector.tensor_add(ob3, po3, s_bcast)
                        nc.sync.dma_start(
                            out[
                                t0 + tt * P : t0 + (tt + 1) * P,
                                ot * nfree : (ot + 1) * nfree,
                            ],
                            ob[:],
                        )
    nc.compile()
    return nc


def _get_nc(reps=1):
    key = ("nc", reps)
    if key not in _NC_CACHE:
        _NC_CACHE[key] = _build_nc(reps)
    return _NC_CACHE[key]


def prep_in_maps(x, weight, bone):
    """Host-side layout prep: transposes + block placement + fp16 cast."""
    x = np.asarray(x)
    weight = np.asarray(weight)
    bone = np.asarray(bone)

    wT16 = np.ascontiguousarray(weight.T).astype(np.float16)
    boneT = bone.transpose(0, 2, 1).astype(np.float16)  # bone[b]^T
    bdmat = np.zeros((KT, P, P), np.float16)
    bdmat[:, 0:R, 0:R] = boneT[0::2]
    bdmat[:, R:P, R:P] = boneT[1::2]
    bd_host = np.ascontiguousarray(bdmat.transpose(1, 0, 2))  # [P, KT, P]
    bdvm = np.zeros((KT, P, R), np.float16)
    bdvm[:, 0:R, :] = boneT[0::2]
    bdvm[:, R:P, :] = boneT[1::2]
    bdv_host = np.ascontiguousarray(bdvm.transpose(1, 0, 2))  # [P, KT, R]

    in_maps = []
    for i in range(B):
        xT16 = np.ascontiguousarray(x[i].T).astype(np.float16)
        in_maps.append({"xT": xT16, "wT": wT16, "bd": bd_host, "bdv": bdv_host})
    return in_maps


def kernel(x, weight, bone):
    from concourse.bass_utils import run_bass_kernel_spmd

    nc = _get_nc()
    in_maps = prep_in_maps(x, weight, bone)
    res = run_bass_kernel_spmd(nc, in_maps, core_ids=list(range(B)))
    return np.stack([r["out"] for r in res.results], axis=0)


if __name__ == "__main__":
    rng = np.random.default_rng(0)
    x = rng.standard_normal((B, T, IN), dtype=np.float32)
    weight = (rng.standard_normal((OUT, IN)) * 0.02).astype(np.float32)
    bone = (rng.standard_normal((IN // R, R, R)) * 0.02).astype(np.float32)
    out = kernel(x=x, weight=weight, bone=bone)
    print(out.shape, out.dtype)


# revision 24
# speedup vs baseline: 4.2544x; 1.0977x over previous
"""BoneLinear Trainium2 kernel (8-core SPMD, data-parallel over batch).

Math: reference computes out = x @ (weight + w)^T where w is the bone
block-update of weight:
    wblk = weight.reshape(a, r, b, r).transpose(0,2,1,3)      # (a,b,r,r)
    wup  = wblk @ bone[b] + bone[b]                            # per (a,b)
    w    = wup.transpose(0,2,1,3).reshape(out_f, in_f)

Identity used here (verified numerically): with y[:, b*r:(b+1)*r] =
x[:, b*r:(b+1)*r] @ bone[b].T and s = sum_b y[:, b-block]:

    out = (x + y) @ weight^T + tile(s over out-blocks)

so the heavy GEMM uses the *original* weight; the bone update reduces to a
cheap block-diagonal transform of x plus a rank-64 broadcast correction.

Per core (batch element): z^T = x^T + blockdiag(bone^T) @ x^T is computed on
the PE in t-quarters and kept SBUF-resident in fp16; the main GEMM
out = z^T.T @ W^T streams W^T from HBM; s is accumulated on the PE and added
(broadcast over 64-column blocks) during PSUM eviction.
"""

import numpy as np

B, T, IN, OUT, R = 8, 2048, 4096, 4096, 64
P = 128
KT = IN // P  # 32 contraction tiles
TQ = 512  # t-quarter size
NQ = T // TQ  # 4 quarters
NFREE = 512  # matmul moving free dim / o-tile size
OTN = OUT // NFREE  # 8 o-tiles

_NC_CACHE = {}


def _build_nc(
    reps=1,
    nfree=NFREE,
    po_bufs=4,
    py_bufs=2,
    xt_bufs=36,
    wt_on_act=True,
    act_copy=True,
    xt_chunk=1,
):
    import concourse.mybir as mybir
    from concourse import bacc
    from concourse.tile import TileContext
    from concourse.masks import make_identity

    F16 = mybir.dt.float16
    F32 = mybir.dt.float32
    otn = OUT // nfree

    nc = bacc.Bacc(None, target_bir_lowering=False)
    xT = nc.dram_tensor("xT", [IN, T], F16, kind="ExternalInput")
    wT = nc.dram_tensor("wT", [IN, OUT], F16, kind="ExternalInput")
    bd = nc.dram_tensor("bd", [P, KT, P], F16, kind="ExternalInput")
    bdv = nc.dram_tensor("bdv", [P, KT, R], F16, kind="ExternalInput")
    out = nc.dram_tensor("out", [T, OUT], F32, kind="ExternalOutput")

    wTv = wT.rearrange("(ko p) o -> p ko o", p=P)

    with TileContext(nc) as tc:
        with (
            tc.tile_pool(name="const", bufs=1) as constp,
            tc.tile_pool(name="xt", bufs=xt_bufs) as xpool,
            tc.tile_pool(name="zt", bufs=2) as zpool,
            tc.tile_pool(name="wt", bufs=2) as wpool,
            tc.tile_pool(name="sb", bufs=2) as spool,
            tc.tile_pool(name="ob", bufs=3) as opool,
            tc.tile_pool(name="py", bufs=py_bufs, space="PSUM") as pyp,
            tc.tile_pool(name="ps", bufs=1, space="PSUM") as psp,
            tc.tile_pool(name="po", bufs=po_bufs, space="PSUM") as pop,
        ):
            bd_sb = constp.tile([P, KT, P], F16, tag="bd")
            nc.sync.dma_start(bd_sb[:], bd[:])
            bdv_sb = constp.tile([P, KT, R], F16, tag="bdv")
            nc.sync.dma_start(bdv_sb[:], bdv[:])
            ident = constp.tile([R, R], F32, tag="ident")
            make_identity(nc, ident)

            # Next-quarter xt tiles are prefetched during the current
            # quarter's phase 2, so phase 1 never waits on DMA (and the small
            # xt loads don't get stuck behind a 4MB wt transfer at the
            # quarter boundary). xt_chunk>1 batches that many k-tiles per
            # DMA (fewer, larger transfers).
            xt_tiles = {}
            xTv = xT.rearrange("(ko p) t -> p ko t", p=P)

            def prefetch_xt(qi):
                tq0 = (qi % NQ) * TQ
                tiles = []
                for kc in range(0, KT, xt_chunk):
                    xt = xpool.tile([P, xt_chunk, TQ], F16, tag="xt")
                    nc.sync.dma_start(
                        xt[:], xTv[:, kc : kc + xt_chunk, tq0 : tq0 + TQ]
                    )
                    for j in range(xt_chunk):
                        tiles.append(xt[:, j, :])
                xt_tiles[qi] = tiles

            # reps>1 repeats the whole computation (timing builds only —
            # wall-time differencing cancels host/transfer overhead).
            for qi in range(NQ * reps):
                q = qi % NQ
                t0 = q * TQ
                if qi == 0:
                    prefetch_xt(0)
                # ---- phase 1: z^T quarter + s accumulation ----
                zt = zpool.tile([P, KT, TQ], F16, tag="zt")
                psum_s = psp.tile([R, TQ], F32, tag="ps")
                xts = xt_tiles.pop(qi)
                for k in range(KT):
                    xt = xts[k]
                    py = pyp.tile([P, TQ], F32, tag="py")
                    # z^T tile directly: blockdiag(I + bone[2k]^T, ...) @ x^T
                    # (x rides through the identity exactly — same rounding
                    # as an explicit fp32 add of fp16 x).
                    nc.tensor.matmul(
                        py[:], bd_sb[:, k, :], xt[:], start=True, stop=True
                    )
                    # s^T accumulation: vstack(bone[2k]^T, bone[2k+1]^T)
                    nc.tensor.matmul(
                        psum_s[:],
                        bdv_sb[:, k, :],
                        xt[:],
                        start=(k == 0),
                        stop=(k == KT - 1),
                    )
                    # psum -> SBUF fp16; split 2:1 across DVE and the idle
                    # ACT engine so the copies keep pace with the PE.
                    if act_copy and k % 3 == 2:
                        nc.scalar.copy(zt[:, k, :], py[:])
                    else:
                        nc.vector.tensor_copy(zt[:, k, :], py[:])
                # s: [R, TQ] -> t-partition layout [P, TQ//P, R]
                sT = spool.tile([R, TQ], F32, tag="sT")
                nc.vector.tensor_copy(sT[:], psum_s[:])
                s_sb = spool.tile([P, TQ // P, R], F32, tag="ssb")
                for c in range(TQ // P):
                    pt = pyp.tile([P, R], F32, tag="py")
                    nc.tensor.transpose(pt[:], sT[:, c * P : (c + 1) * P], ident[:])
                    nc.vector.tensor_copy(s_sb[:, c, :], pt[:])

                # ---- phase 2: out quarter = z^T.T @ W^T + s ----
                # wt DMAs ride the ACT HWDGE ring (wt_on_act) so the next
                # quarter's first weight tile isn't FIFO-queued behind the 32
                # xt loads on the SP ring — hides the 4MB load under compute.
                wt_dma = nc.scalar.dma_start if wt_on_act else nc.sync.dma_start
                for ot in range(otn):
                    wt = wpool.tile([P, KT, nfree], F16, tag="wt")
                    wt_dma(wt[:], wTv[:, :, ot * nfree : (ot + 1) * nfree])
                    if ot == 0 and qi + 1 < NQ * reps:
                        # Next quarter's x loads: emitted after this quarter's
                        # first weight tile so the SP ring serves wt first.
                        prefetch_xt(qi + 1)
                    for tt in range(TQ // P):
                        po = pop.tile([P, nfree], F32, tag="po")
                        for k in range(KT):
                            nc.tensor.matmul(
                                po[:],
                                zt[:, k, tt * P : (tt + 1) * P],
                                wt[:, k, :],
                                start=(k == 0),
                                stop=(k == KT - 1),
                            )
                        ob = opool.tile([P, nfree], F32, tag="ob")
                        ob3 = ob.rearrange("p (a r) -> p a r", r=R)
                        po3 = po.rearrange("p (a r) -> p a r", r=R)
                        s_bcast = s_sb[:, tt, :][:, None, :].to_broadcast(
                            (P, nfree // R, R)
                        )
                        nc.vector.tensor_add(ob3, po3, s_bcast)
                        nc.sync.dma_start(
                            out[
                                t0 + tt * P : t0 + (tt + 1) * P,
                                ot * nfree : (ot + 1) * nfree,
                            ],
                            ob[:],
                        )
    nc.compile()
    return nc


def _get_nc(reps=1):
    key = ("nc", reps)
    if key not in _NC_CACHE:
        _NC_CACHE[key] = _build_nc(reps)
    return _NC_CACHE[key]


def prep_in_maps(x, weight, bone):
    """Host-side layout prep: transposes + block placement + fp16 cast."""
    x = np.asarray(x, dtype=np.float32)
    weight = np.asarray(weight, dtype=np.float32)
    bone = np.asarray(bone, dtype=np.float32)
    assert x.shape == (B, T, IN), x.shape
    assert weight.shape == (OUT, IN), weight.shape
    assert bone.shape == (IN // R, R, R), bone.shape

    wT16 = np.ascontiguousarray(weight.T).astype(np.float16)
    boneT = bone.transpose(0, 2, 1).astype(np.float16)  # bone[b]^T
    bdmat = np.zeros((KT, P, P), np.float16)
    bdmat[:, 0:R, 0:R] = boneT[0::2]
    bdmat[:, R:P, R:P] = boneT[1::2]
    bdmat += np.eye(P, dtype=np.float16)[None]  # fold the +x into the y-mm
    bd_host = np.ascontiguousarray(bdmat.transpose(1, 0, 2))  # [P, KT, P]
    bdvm = np.zeros((KT, P, R), np.float16)
    bdvm[:, 0:R, :] = boneT[0::2]
    bdvm[:, R:P, :] = boneT[1::2]
    bdv_host = np.ascontiguousarray(bdvm.transpose(1, 0, 2))  # [P, KT, R]

    in_maps = []
    for i in range(B):
        xT16 = np.ascontiguousarray(x[i].T).astype(np.float16)
        in_maps.append({"xT": xT16, "wT": wT16, "bd": bd_host, "bdv": bdv_host})
    return in_maps


def kernel(x, weight, bone):
    from concourse.bass_utils import run_bass_kernel_spmd

    nc = _get_nc()
    in_maps = prep_in_maps(x, weight, bone)
    res = run_bass_kernel_spmd(nc, in_maps, core_ids=list(range(B)))
    return np.stack([r["out"] for r in res.results], axis=0)


if __name__ == "__main__":
    rng = np.random.default_rng(0)
    x = rng.standard_normal((B, T, IN), dtype=np.float32)
    weight = (rng.standard_normal((OUT, IN)) * 0.02).astype(np.float32)
    bone = (rng.standard_normal((IN // R, R, R)) * 0.02).astype(np.float32)
    out = kernel(x=x, weight=weight, bone=bone)
    print(out.shape, out.dtype)
